# revision 1
# baseline (speedup 1.0000x reference)
"""Trainium2 Bass kernel for nn_Eva_62818191671438 (sparse_attention).

Sharding: 8 cores = (batch b in 0..3) x (head-half in 0..1); each core runs
QKV projection + attention for its 8 heads + partial output projection
(row-parallel TP); host sums the two partials per batch and adds proj_b.

Structural optimizations:
- The T=2 per-head-token attentions share the big S x S spatial
  logits/exp/AV work; only rank-1 corrections differ per token.
- Softmax uses plain exp (logits ~N(0,1): no overflow risk) with the
  denominator obtained free via a ones-column appended to V.
- RoPE's interleaved rotation becomes two contiguous partition-block
  multiplies after an evens-then-odds permutation of each head's D axis
  (folded into the QKV weights host-side).
Precision strategy (all matmuls on the PE at bf16-class rates):
- QKV runs as a 3-term bf16 hi/lo product (x and W split host-side),
  q/k are stored float32r so logits carry tf32-class accuracy into exp.
- Attention weights/values (E, V) and the output projection are bf16;
  the projection uses an oT/proj_w hi/lo 3-term product to avoid
  eps-level dot-product error.
"""
import sys
sys.path.insert(0, "/opt/trn_rl_repo")
import numpy as np

B, N, C, H, T, D, S = 4, 1026, 1024, 16, 2, 64, 1024
HPC = 8          # heads per core
NCORES = 8
KC = 8           # contraction chunks (C/128)
QCW = 342        # attention query-chunk width (3*342 = 1026)
PN = 1028        # padded N for 4x257 partition packing
PW = 257         # packed chunk width
SCALE = D ** -0.5

_CACHE = {}


def _build_nc(shuffle_engine="vector", debug=False, dump=False, repeat=1,
              serial=False):
    import concourse.bacc as bacc
    import concourse.mybir as mybir
    import concourse.tile as tile

    F32 = mybir.dt.float32
    F32R = mybir.dt.float32r
    MDT = mybir.dt.bfloat16
    AF = mybir.ActivationFunctionType
    MUL = mybir.AluOpType.mult
    ADD = mybir.AluOpType.add

    nc = bacc.Bacc("TRN2", target_bir_lowering=False, debug=debug,
                   num_devices=NCORES)
    xTh = nc.dram_tensor("xTh", [1025, N], MDT, kind="ExternalInput").ap()
    xTl = nc.dram_tensor("xTl", [1025, N], MDT, kind="ExternalInput").ap()
    wTh = nc.dram_tensor("wTh", [1025, 1536], MDT, kind="ExternalInput").ap()
    wTl = nc.dram_tensor("wTl", [1025, 1536], MDT, kind="ExternalInput").ap()
    pjTh = nc.dram_tensor("pjTh", [512, 1024], MDT, kind="ExternalInput").ap()
    pjTl = nc.dram_tensor("pjTl", [512, 1024], MDT, kind="ExternalInput").ap()
    cosP = nc.dram_tensor("cosP", [128, S], F32, kind="ExternalInput").ap()
    sinA = nc.dram_tensor("sinA", [128, S], F32, kind="ExternalInput").ap()
    outs = [nc.dram_tensor("out" if r == 0 else f"out{r}", [N, C], F32,
                            kind="ExternalOutput").ap() for r in range(repeat)]
    serial_prev = [None]

    dbg = {}
    if dump:
        for nm, shp in [("d_qT", [128, 4, N]), ("d_kT", [128, 4, N]),
                        ("d_avsb", [65, HPC, PN]), ("d_zb", [32, 3, PW]),
                        ("d_sgs", [32, PW]), ("d_sgg0", [32, PW]),
                        ("d_oT", [128, 4, N]), ("d_vz", [128, KC, HPC, 65])]:
            dt_ = (F32 if nm in ("d_avsb", "d_zb", "d_sgs")
                   else (F32R if nm in ("d_qT", "d_kT", "d_sgg0") else MDT))
            dbg[nm] = nc.dram_tensor(nm, shp, dt_, kind="ExternalOutput").ap()
    with tile.TileContext(nc) as tc:
        for rep in range(repeat):
            _emit(nc, tc, tile, mybir, F32, MDT, AF, MUL, ADD,
                  xTh, xTl, wTh, wTl, pjTh, pjTl, cosP, sinA, outs[rep],
                  shuffle_engine,
                  dbg if rep == 0 else {}, sfx=str(rep) if repeat > 1 else "",
                  chain=(outs[rep - 1] if (serial and rep > 0) else None))
    nc.compile()
    return nc


def _emit(nc, tc, tile, mybir, F32, MDT, AF, MUL, ADD,
          xTh, xTl, wTh, wTl, pjTh, pjTl, cosP, sinA, out, shuffle_engine,
          dbg={}, sfx="", chain=None):
    import os as _os
    safe = _os.environ.get("CORESIM_SAFE", "0") == "1"
    F32R = mybir.dt.float32r
    from contextlib import ExitStack
    ctx = ExitStack()
    with ctx:
        pm = ctx.enter_context(tc.tile_pool(name="pm" + sfx, bufs=1))
        psum = ctx.enter_context(tc.tile_pool(name="psum" + sfx, bufs=1, space="PSUM"))

        # ---------------- persistent tiles ----------------
        qT = pm.tile([128, 4, N], F32R, name="qT")
        kT = pm.tile([128, 4, N], F32R, name="kT")
        vz = pm.tile([128, KC, HPC, 65], MDT, name="vz")
        vtok0 = pm.tile([1, HPC, 65], F32R, name="vtok0")
        vtok1 = pm.tile([1, HPC, 65], F32R, name="vtok1")
        oTh = pm.tile([128, 4, N], MDT, name="oTh")
        oTl = pm.tile([128, 4, N], MDT, name="oTl")
        ones64 = pm.tile([128, 64], F32, name="ones64")
        if chain is not None:
            chaint = pm.tile([1, C], F32, name="chaint")
            nc.sync.dma_start(chaint[:], chain[0:1, :])
            nc.vector.tensor_copy(ones64[0:1, 0:1], chaint[0:1, 0:1])
        nc.gpsimd.memset(ones64[:], 1.0)
        nc.vector.tensor_copy(
            vz[:, :, :, 64:65],
            ones64[:].rearrange("p (a b o) -> p a b o", a=KC, b=HPC))


        # ---------------- stage A+B: load + QKV ----------------
        with tc.tile_pool(name="pw" + sfx, bufs=1) as pw, \
             tc.tile_pool(name="pr" + sfx, bufs=3) as pr:
            xtsh = pw.tile([128, KC, N], MDT, name="xtsh")
            xtsl = pw.tile([128, KC, N], MDT, name="xtsl")
            xone = pw.tile([1, N], MDT, name="xone")
            wtsh = pw.tile([128, KC, 1536], MDT, name="wtsh")
            wtsl = pw.tile([128, KC, 1536], MDT, name="wtsl")
            wbh = pw.tile([1, 1536], MDT, name="wbh")
            wbl = pw.tile([1, 1536], MDT, name="wbl")
            cosPt = pw.tile([128, S], F32, name="cosPt")
            sinAt = pw.tile([128, S], F32, name="sinAt")
            nc.sync.dma_start(cosPt[:], cosP[:])
            nc.sync.dma_start(sinAt[:], sinA[:])
            nc.sync.dma_start(
                xtsh[:], xTh[0:1024].rearrange("(k p) n -> p k n", p=128))
            nc.sync.dma_start(
                xtsl[:], xTl[0:1024].rearrange("(k p) n -> p k n", p=128))
            nc.sync.dma_start(xone[:], xTh[1024:1025])
            nc.sync.dma_start(
                wtsh[:], wTh[0:1024].rearrange("(k p) n -> p k n", p=128))
            nc.sync.dma_start(
                wtsl[:], wTl[0:1024].rearrange("(k p) n -> p k n", p=128))
            nc.sync.dma_start(wbh[:], wTh[1024:1025])
            nc.sync.dma_start(wbl[:], wTl[1024:1025])

            # q/k groups: out [dout 128, n-chunk]; rope on spatial chunks
            FQK = [(0, 2), (2, 514), (514, 1026)]
            for qk in range(2):          # 0=q, 1=k
                dst = qT if qk == 0 else kT
                for g in range(4):
                    gc = qk * 512 + g * 128
                    for (f0, f1) in FQK:
                        fw = f1 - f0
                        ps = psum.tile([128, 512], F32, tag="lgt", name="psqk", bufs=2)
                        first = True
                        for (wop, xop) in ((wtsh, xtsh), (wtsh, xtsl),
                                           (wtsl, xtsh)):
                            for kc in range(KC):
                                nc.tensor.matmul(
                                    ps[:, :fw], wop[:, kc, gc:gc + 128],
                                    xop[:, kc, f0:f1], start=first, stop=False)
                                first = False
                        nc.tensor.matmul(
                            ps[:, :fw], wbh[:, gc:gc + 128], xone[:, f0:f1],
                            start=False, stop=True)
                        if f0 == 0:      # token cols: no rope, plain copy
                            nc.scalar.activation(dst[:, g, 0:2], ps[:, :2],
                                                 AF.Identity)
                            continue
                        sl = slice(f0 - 2, f1 - 2)
                        tmp = pr.tile([128, 512], F32, tag="rtmp", name="rtmp")
                        cq = pr.tile([128, 512], F32, tag="rcq", name="rcq")
                        sh = nc.gpsimd if shuffle_engine == "gpsimd" else nc.vector
                        for hb in (0, 64):   # two heads per chunk
                            sh.tensor_tensor(
                                out=tmp[hb:hb + 32, :], in0=ps[hb + 32:hb + 64, :],
                                in1=sinAt[hb:hb + 32, sl], op=MUL)
                            sh.tensor_tensor(
                                out=tmp[hb + 32:hb + 64, :], in0=ps[hb:hb + 32, :],
                                in1=sinAt[hb + 32:hb + 64, sl], op=MUL)
                        nc.vector.tensor_tensor(
                            out=cq[:], in0=ps[:, :fw], in1=cosPt[:, sl], op=MUL)
                        nc.vector.tensor_tensor(
                            out=dst[:, g, f0:f1], in0=cq[:], in1=tmp[:], op=ADD)

            # v groups: out [n-chunk, dout 512]
            NSL = [(0, 2)] + [(2 + 128 * i, 2 + 128 * (i + 1)) for i in range(8)]
            for si, (n0, n1) in enumerate(NSL):
                nw = n1 - n0
                ps = psum.tile([128, 512], F32, tag="lgt", name="psv", bufs=2)
                for kc in range(KC):
                    nc.tensor.matmul(ps[:nw, :], xtsh[:, kc, n0:n1],
                                     wtsh[:, kc, 1024:1536],
                                     start=(kc == 0), stop=False)
                nc.tensor.matmul(ps[:nw, :], xone[:, n0:n1], wbh[:, 1024:1536],
                                 start=False, stop=True)
                if si == 0:
                    vtk2 = pr.tile([2, HPC, 64], F32R, tag="vtk2", name="vtk2")
                    nc.scalar.activation(
                        vtk2[:],
                        ps[0:2, :].rearrange("p (h d) -> p h d", h=HPC),
                        AF.Identity)
                    nc.sync.dma_start(vtok0[:, :, 0:64], vtk2[0:1])
                    nc.sync.dma_start(vtok1[:, :, 0:64], vtk2[1:2])
                else:
                    nc.scalar.activation(
                        vz[:, si - 1, :, 0:64],
                        ps[:, :].rearrange("p (h d) -> p h d", h=HPC),
                        AF.Identity)

        if dbg:
            nc.sync.dma_start(dbg["d_qT"], qT[:])
            nc.sync.dma_start(dbg["d_kT"], kT[:])
            nc.sync.dma_start(dbg["d_vz"], vz[:])
        # ---------------- stage C: attention ----------------
        with tc.tile_pool(name="pc" + sfx, bufs=1) as pc, \
             tc.tile_pool(name="pe2" + sfx, bufs=2) as pe2:
            zt = pc.tile([32, PW], F32, name="zt")
            e0t = pc.tile([32, PW], F32, name="e0t")
            e1t = pc.tile([32, PW], F32, name="e1t")
            r0t = pc.tile([32, PW], F32, name="r0t")
            r1t = pc.tile([32, PW], F32, name="r1t")
            sgs = pc.tile([32, PW], F32, name="sgs")
            sgg0 = pc.tile([32, PW], F32R, name="sgg0")
            sgg1 = pc.tile([32, PW], F32R, name="sgg1")
            avsb = pc.tile([65, HPC, PN], F32, name="avsb")
            nc.gpsimd.memset(avsb[:, :, 1026:PN], 1.0)

            for h in range(HPC):
                j, po = h // 2, (h % 2) * 64
                etokh = pe2.tile([2, PN], F32, tag="etokh", name="etokh")
                nc.vector.memset(etokh[:, 1026:PN], 1.0)
                for qc in range(3):
                    q0 = qc * QCW
                    qsl = slice(q0, q0 + QCW)
                    et = pe2.tile([128, KC, QCW], MDT, tag="et", name="et", bufs=4)
                    for ktg in range(4):     # kt pairs
                        lg = psum.tile([128, 2, 512], F32, tag="lgt", name="lg", bufs=2)
                        for u in range(2):
                            kt = 2 * ktg + u
                            ksl = slice(2 + kt * 128, 2 + (kt + 1) * 128)
                            nc.tensor.matmul(
                                lg[:, u, :QCW], kT[po:po + 64, j, ksl],
                                qT[po:po + 64, j, qsl], start=True, stop=True)
                        nc.scalar.activation(
                            et[:, 2 * ktg:2 * ktg + 2, :], lg[:, :, :QCW],
                            AF.Exp, scale=SCALE)
                    av = psum.tile([65, 512], F32, tag="av", name="av", bufs=3)
                    for kt in range(KC):
                        nc.tensor.matmul(av[:, :QCW], vz[:, kt, h, :],
                                         et[:, kt, :], start=(kt == 0),
                                         stop=(kt == KC - 1))
                    pstokh = psum.tile([2, 512], F32, tag="tok",
                                       name="pstokh")
                    nc.tensor.matmul(
                        pstokh[:, 0:QCW], kT[po:po + 64, j, 0:2],
                        qT[po:po + 64, j, qsl], start=True, stop=True)
                    nc.scalar.activation(etokh[:, qsl], pstokh[:, 0:QCW],
                                         AF.Exp, scale=SCALE)
                    # evacuate av (A rows + Z row) to SBUF in one op
                    nc.scalar.activation(avsb[:, h, qsl], av[0:65, :QCW],
                                         AF.Identity)
                if safe:
                    for c4 in range(4):
                        nc.sync.dma_start(zt[h + 8 * c4:h + 8 * c4 + 1, :],
                                          avsb[64:65, h, PW * c4:PW * (c4 + 1)])
                else:
                    nc.sync.dma_start(zt[h::8, :], avsb[64:65, h, :])
                if safe:
                    for c4 in range(4):
                        nc.sync.dma_start(e0t[h + 8 * c4:h + 8 * c4 + 1, :],
                                          etokh[0:1, PW * c4:PW * (c4 + 1)])
                        nc.sync.dma_start(e1t[h + 8 * c4:h + 8 * c4 + 1, :],
                                          etokh[1:2, PW * c4:PW * (c4 + 1)])
                else:
                    nc.sync.dma_start(e0t[h::8, :], etokh[0:1, :])
                    nc.sync.dma_start(e1t[h::8, :], etokh[1:2, :])

            # batched corrections (all heads, full width)
            nc.vector.tensor_tensor(out=r0t[:], in0=e0t[:], in1=zt[:], op=ADD)
            nc.vector.tensor_tensor(out=r1t[:], in0=e1t[:], in1=zt[:], op=ADD)
            nc.vector.reciprocal(r0t[:], r0t[:])
            nc.vector.reciprocal(r1t[:], r1t[:])
            sc = pc.tile([32, PW], F32, name="sc")
            nc.vector.tensor_tensor(out=sc[:], in0=r0t[:], in1=r1t[:], op=ADD)
            nc.vector.tensor_scalar_mul(sgs[:], sc[:], 0.5)
            nc.vector.tensor_tensor(out=sc[:], in0=e0t[:], in1=r0t[:], op=MUL)
            nc.vector.tensor_scalar_mul(sgg0[:], sc[:], 0.5)
            nc.vector.tensor_tensor(out=sc[:], in0=e1t[:], in1=r1t[:], op=MUL)
            nc.vector.tensor_scalar_mul(sgg1[:], sc[:], 0.5)
            # token-query fixups (cols 0,1): full r_t / e00*r_t, zero cross-t
            for t, (sgg, rbs, zbs) in enumerate(
                    [(sgg0, r0t, e0t), (sgg1, r1t, e1t)]):
                tc_ = slice(t, t + 1)
                nc.vector.tensor_copy(sgs[0:8, tc_], rbs[0:8, tc_])
                nc.vector.tensor_tensor(out=sgg[0:8, tc_], in0=zbs[0:8, tc_],
                                        in1=rbs[0:8, tc_], op=MUL)
                oth = sgg1 if t == 0 else sgg0
                nc.vector.tensor_scalar_mul(oth[0:8, tc_], oth[0:8, tc_], 0.0)

            # final combine per head
            for h in range(HPC):
                j, po = h // 2, (h % 2) * 64
                sgsr = pe2.tile([1, PN], F32, tag="sgsr", name="sgsr")
                if safe:
                    for c4 in range(4):
                        nc.sync.dma_start(sgsr[0:1, PW * c4:PW * (c4 + 1)],
                                          sgs[h + 8 * c4:h + 8 * c4 + 1, :])
                else:
                    nc.sync.dma_start(sgsr[:], sgs[h::8, :])
                bc = pe2.tile([64, PN], F32, tag="bc", name="bc")
                nc.gpsimd.partition_broadcast(bc[:], sgsr[0:1, :])
                sgr0 = pe2.tile([1, PN], F32R, tag="sgr0", name="sgr0")
                sgr1 = pe2.tile([1, PN], F32R, tag="sgr1", name="sgr1")
                if safe:
                    for c4 in range(4):
                        nc.sync.dma_start(sgr0[0:1, PW * c4:PW * (c4 + 1)],
                                          sgg0[h + 8 * c4:h + 8 * c4 + 1, :])
                        nc.sync.dma_start(sgr1[0:1, PW * c4:PW * (c4 + 1)],
                                          sgg1[h + 8 * c4:h + 8 * c4 + 1, :])
                else:
                    nc.sync.dma_start(sgr0[:], sgg0[h::8, :])
                    nc.sync.dma_start(sgr1[:], sgg1[h::8, :])
                for qc in range(3):
                    q0 = qc * QCW
                    qsl = slice(q0, q0 + QCW)
                    vt = psum.tile([64, 512], F32, tag="av", name="vt", bufs=3)
                    nc.tensor.matmul(vt[:, :QCW], vtok0[:, h, :64],
                                     sgr0[:, qsl], start=True, stop=False)
                    nc.tensor.matmul(vt[:, :QCW], vtok1[:, h, :64],
                                     sgr1[:, qsl], start=False, stop=True)
                    tf = pe2.tile([64, 512], F32, tag="tf", name="tf")
                    of = pe2.tile([64, 512], F32, tag="of", name="of")
                    nc.vector.tensor_tensor(out=tf[:, :QCW],
                                            in0=avsb[0:64, h, qsl],
                                            in1=bc[:, qsl], op=MUL)
                    nc.vector.tensor_tensor(out=of[:, :QCW], in0=vt[:, :QCW],
                                            in1=tf[:, :QCW], op=ADD)
                    ohs = pe2.tile([64, 512], MDT, tag="ohs", name="ohs")
                    nc.vector.tensor_copy(ohs[:, :QCW], of[:, :QCW])
                    nc.vector.tensor_tensor(out=oTl[po:po + 64, j, qsl],
                                            in0=of[:, :QCW], in1=ohs[:, :QCW],
                                            op=mybir.AluOpType.subtract)
                    nc.vector.tensor_copy(oTh[po:po + 64, j, qsl],
                                          ohs[:, :QCW])

            if dbg:
                nc.sync.dma_start(dbg["d_avsb"], avsb[:])
                nc.sync.dma_start(dbg["d_zb"][:, 0], zt[:])
                nc.sync.dma_start(dbg["d_zb"][:, 1], e0t[:])
                nc.sync.dma_start(dbg["d_zb"][:, 2], e1t[:])
                nc.sync.dma_start(dbg["d_sgs"], sgs[:])
                nc.sync.dma_start(dbg["d_sgg0"], sgg0[:])
        if dbg:
            nc.sync.dma_start(dbg["d_oT"], oTh[:])
        # ---------------- stage D: output projection ----------------
        with tc.tile_pool(name="pd" + sfx, bufs=1) as pd, \
             tc.tile_pool(name="po2" + sfx, bufs=3) as po2:
            pjtsh = pd.tile([128, 4, 1024], MDT, name="pjtsh")
            pjtsl = pd.tile([128, 4, 1024], MDT, name="pjtsl")
            nc.sync.dma_start(
                pjtsh[:], pjTh.rearrange("(k p) n -> p k n", p=128))
            nc.sync.dma_start(
                pjtsl[:], pjTl.rearrange("(k p) n -> p k n", p=128))
            NSL = [(128 * i, 128 * (i + 1)) for i in range(8)] + [(1024, 1026)]
            for (n0, n1) in NSL:
                nw = n1 - n0
                for f in range(2):
                    ps = psum.tile([128, 512], F32, tag="lgt", name="psp", bufs=2)
                    first = True
                    for (oo, pp) in ((oTh, pjtsh), (oTh, pjtsl), (oTl, pjtsh)):
                        for j in range(4):
                            nc.tensor.matmul(
                                ps[:nw, :], oo[:, j, n0:n1],
                                pp[:, j, 512 * f:512 * (f + 1)],
                                start=first, stop=False)
                            first = False
                    nc.tensor.matmul(ps[:nw, :], oTl[:, 3, n0:n1],
                                     pjtsl[:, 3, 512 * f:512 * (f + 1)],
                                     start=False, stop=True)
                    ob = po2.tile([128, 512], F32, tag="ob", name="ob")
                    nc.vector.tensor_copy(ob[:nw, :], ps[:nw, :])
                    nc.sync.dma_start(out[n0:n1, 512 * f:512 * (f + 1)],
                                      ob[:nw, :])


# ---------------- host side ----------------

_PERM = np.concatenate([np.arange(0, 64, 2), np.arange(1, 64, 2)])


def _host_prep(x, rope, qkv_w, q_bias, k_bias, v_bias, proj_w):
    """Build per-core input dicts."""
    x = np.asarray(x, np.float32)
    rope = np.asarray(rope, np.float32)
    qkv_w = np.asarray(qkv_w, np.float32)
    q_bias = np.asarray(q_bias, np.float32)
    k_bias = np.asarray(k_bias, np.float32)
    v_bias = np.asarray(v_bias, np.float32)
    proj_w = np.asarray(proj_w, np.float32)

    sin = rope[:, :D].T          # [64, S]
    cos = rope[:, D:].T
    cos64 = cos[_PERM]
    sinA64 = np.empty((64, S), np.float32)
    sinA64[0:32] = -sin[0::2]
    sinA64[32:64] = sin[1::2]
    cosP = np.vstack([cos64, cos64]).astype(np.float32)
    sinA = np.vstack([sinA64, sinA64]).astype(np.float32)

    in_maps = []
    for core in range(NCORES):
        b, hh = core // 2, core % 2
        hs = hh * 512
        idx = np.concatenate([h * 64 + _PERM for h in range(HPC)]) + hs
        wq = qkv_w[0:C][idx]
        wk = qkv_w[C:2 * C][idx]
        wv = qkv_w[2 * C:3 * C][hs:hs + 512]
        W3 = np.concatenate([wq, wk, wv], 0)          # [1536, 1024]
        wT = np.empty((1025, 1536), np.float32)
        wT[0:1024] = W3.T
        wT[1024] = np.concatenate(
            [q_bias[idx], k_bias[idx], v_bias[hs:hs + 512]])
        xTa = np.empty((1025, N), np.float32)
        xTa[0:1024] = x[b].T
        xTa[1024] = 1.0
        pjT = np.ascontiguousarray(proj_w[:, hs:hs + 512].T)  # [512, 1024]
        import ml_dtypes
        bf = ml_dtypes.bfloat16
        xh = xTa.astype(bf)
        xl = (xTa - xh.astype(np.float32)).astype(bf)
        wh = wT.astype(bf)
        wl = (wT - wh.astype(np.float32)).astype(bf)
        pjh = pjT.astype(bf)
        pjl = (pjT - pjh.astype(np.float32)).astype(bf)
        in_maps.append({"xTh": xh, "xTl": xl, "wTh": wh, "wTl": wl,
                        "pjTh": pjh, "pjTl": pjl,
                        "cosP": cosP, "sinA": sinA})
    return in_maps


def _get_runner():
    return _get_runner_rep(1)


def _get_runner_rep(repeat, serial=False):
    key = f"runner{repeat}s{int(serial)}" 
    if key in _CACHE:
        return _CACHE[key]
    import jax
    from jax.sharding import Mesh, PartitionSpec
    from jax.experimental.shard_map import shard_map
    import concourse.mybir as mybir
    from concourse import bass2jax

    nc = _build_nc(repeat=repeat, serial=serial)
    bass2jax.install_neuronx_cc_hook()
    in_names, out_names, out_avals = [], [], []
    partition_name = (nc.partition_id_tensor.name
                      if nc.partition_id_tensor else None)
    for alloc in nc.m.functions[0].allocations:
        if not isinstance(alloc, mybir.MemoryLocationSet):
            continue
        name = alloc.memorylocations[0].name
        if alloc.kind == "ExternalInput":
            if name != partition_name:
                in_names.append(name)
        elif alloc.kind == "ExternalOutput":
            out_names.append(name)
            out_avals.append(jax.core.ShapedArray(
                tuple(alloc.tensor_shape), mybir.dt.np(alloc.dtype)))
    all_in = list(in_names) + list(out_names)
    if partition_name is not None:
        all_in.append(partition_name)

    def _body(*args):
        operands = list(args)
        if partition_name is not None:
            operands.append(bass2jax.partition_id_tensor())
        return tuple(bass2jax._bass_exec_p.bind(
            *operands, out_avals=tuple(out_avals), in_names=tuple(all_in),
            out_names=tuple(out_names), lowering_input_output_aliases=(),
            sim_require_finite=True, sim_require_nnan=True, nc=nc))

    mesh = Mesh(np.asarray(jax.devices()[:NCORES]), ("core",))
    nin = len(in_names)
    nout = len(out_names)
    fn = jax.jit(
        shard_map(_body, mesh=mesh,
                  in_specs=(PartitionSpec("core"),) * (nin + nout),
                  out_specs=(PartitionSpec("core"),) * nout,
                  check_rep=False),
        keep_unused=True)
    _CACHE[key] = (fn, mesh, in_names, out_names, out_avals)
    return _CACHE[key]


def kernel(x, rope, qkv_w, q_bias, k_bias, v_bias, proj_w, proj_b):
    import jax
    from jax.sharding import PartitionSpec
    fn, mesh, in_names, out_names, out_avals = _get_runner()
    in_maps = _host_prep(x, rope, qkv_w, q_bias, k_bias, v_bias, proj_w)
    sharding = jax.sharding.NamedSharding(mesh, PartitionSpec("core"))
    args = []
    for name in in_names:
        cat = np.concatenate([m[name] for m in in_maps], axis=0)
        args.append(jax.device_put(cat, sharding))
    for av in out_avals:
        z = np.zeros((NCORES * av.shape[0], *av.shape[1:]), av.dtype)
        args.append(jax.device_put(z, sharding))
    outs = fn(*args)
    parts = np.asarray(outs[out_names.index("out")]).reshape(
        NCORES, N, C)
    proj_b = np.asarray(proj_b, np.float32)
    res = np.empty((B, N, C), np.float32)
    for b in range(B):
        res[b] = parts[2 * b] + parts[2 * b + 1] + proj_b
    return res



# revision 3
# speedup vs baseline: 1.1432x; 1.1432x over previous
"""Trainium2 Bass kernel for nn_Eva_62818191671438 (sparse_attention).

Sharding: 8 cores = (batch b in 0..3) x (head-half in 0..1); each core runs
QKV projection + attention for its 8 heads + partial output projection
(row-parallel TP); host sums the two partials per batch and adds proj_b.

Structural optimizations:
- The T=2 per-head-token attentions share the big S x S spatial
  logits/exp/AV work; only rank-1 corrections differ per token.
- Softmax uses plain exp (logits ~N(0,1): no overflow risk) with the
  denominator obtained free via a ones-column appended to V.
- RoPE's interleaved rotation becomes two contiguous partition-block
  multiplies after an evens-then-odds permutation of each head's D axis
  (folded into the QKV weights host-side).
Precision strategy: all matmuls run single-pass in float32r (tf32-class,
full PE rate at free>=256); attention weights/values (E, V) are bf16.
"""
import sys
sys.path.insert(0, "/opt/trn_rl_repo")
import numpy as np

B, N, C, H, T, D, S = 4, 1026, 1024, 16, 2, 64, 1024
HPC = 8          # heads per core
NCORES = 8
KC = 8           # contraction chunks (C/128)
QCW = 342        # attention query-chunk width (3*342 = 1026)
PN = 1028        # padded N for 4x257 partition packing
PW = 257         # packed chunk width
SCALE = D ** -0.5

_CACHE = {}


def _build_nc(shuffle_engine="vector", debug=False, dump=False, repeat=1,
              serial=False):
    import concourse.bacc as bacc
    import concourse.mybir as mybir
    import concourse.tile as tile

    F32 = mybir.dt.float32
    F32R = mybir.dt.float32r
    MDT = mybir.dt.bfloat16
    AF = mybir.ActivationFunctionType
    MUL = mybir.AluOpType.mult
    ADD = mybir.AluOpType.add

    nc = bacc.Bacc("TRN2", target_bir_lowering=False, debug=debug,
                   num_devices=NCORES)
    xT = nc.dram_tensor("xT", [1025, N], F32R, kind="ExternalInput").ap()
    wT = nc.dram_tensor("wT", [1025, 1536], F32R, kind="ExternalInput").ap()
    pjT = nc.dram_tensor("pjT", [512, 1024], F32R, kind="ExternalInput").ap()
    cosP = nc.dram_tensor("cosP", [128, S], F32, kind="ExternalInput").ap()
    sinA = nc.dram_tensor("sinA", [128, S], F32, kind="ExternalInput").ap()
    outs = [nc.dram_tensor("out" if r == 0 else f"out{r}", [N, C], F32,
                            kind="ExternalOutput").ap() for r in range(repeat)]

    dbg = {}
    if dump:
        for nm, shp in [("d_qT", [128, 4, N]), ("d_kT", [128, 4, N]),
                        ("d_avsb", [65, HPC, PN]), ("d_zb", [32, 3, PW]),
                        ("d_sgs", [32, PW]), ("d_sgg0", [32, PW]),
                        ("d_oT", [128, 4, N]), ("d_vz", [128, KC, HPC, 65])]:
            dt_ = (F32 if nm in ("d_avsb", "d_zb", "d_sgs")
                   else (F32R if nm in ("d_qT", "d_kT", "d_sgg0", "d_oT")
                         else MDT))
            dbg[nm] = nc.dram_tensor(nm, shp, dt_, kind="ExternalOutput").ap()
    with tile.TileContext(nc) as tc:
        for rep in range(repeat):
            _emit(nc, tc, tile, mybir, F32, MDT, AF, MUL, ADD,
                  xT, wT, pjT, cosP, sinA, outs[rep],
                  shuffle_engine,
                  dbg if rep == 0 else {}, sfx=str(rep) if repeat > 1 else "",
                  chain=(outs[rep - 1] if (serial and rep > 0) else None))
    nc.compile()
    return nc


def _emit(nc, tc, tile, mybir, F32, MDT, AF, MUL, ADD,
          xT, wT, pjT, cosP, sinA, out, shuffle_engine,
          dbg={}, sfx="", chain=None):
    import os as _os
    safe = _os.environ.get("CORESIM_SAFE", "0") == "1"
    F32R = mybir.dt.float32r
    from contextlib import ExitStack
    ctx = ExitStack()
    with ctx:
        pm = ctx.enter_context(tc.tile_pool(name="pm" + sfx, bufs=1))
        psum = ctx.enter_context(tc.tile_pool(name="psum" + sfx, bufs=1, space="PSUM"))

        # ---------------- persistent tiles ----------------
        qT = pm.tile([128, 4, N], F32R, name="qT")
        kT = pm.tile([128, 4, N], F32R, name="kT")
        vz = pm.tile([128, KC, HPC, 65], MDT, name="vz")
        vtok0 = pm.tile([1, HPC, 65], F32R, name="vtok0")
        vtok1 = pm.tile([1, HPC, 65], F32R, name="vtok1")
        oT = pm.tile([128, 4, N], F32R, name="oT")
        ones64 = pm.tile([128, 64], F32, name="ones64")
        if chain is not None:
            chaint = pm.tile([1, C], F32, name="chaint")
            nc.sync.dma_start(chaint[:], chain[0:1, :])
            nc.vector.tensor_copy(ones64[0:1, 0:1], chaint[0:1, 0:1])
        nc.gpsimd.memset(ones64[:], 1.0)
        nc.vector.tensor_copy(
            vz[:, :, :, 64:65],
            ones64[:].rearrange("p (a b o) -> p a b o", a=KC, b=HPC))


        # ---------------- stage A+B: load + QKV ----------------
        with tc.tile_pool(name="pw" + sfx, bufs=1) as pw, \
             tc.tile_pool(name="pr" + sfx, bufs=3) as pr:
            xts = pw.tile([128, KC, N], F32R, name="xts")
            xone = pw.tile([1, N], F32R, name="xone")
            wts = pw.tile([128, KC, 1536], F32R, name="wts")
            wb = pw.tile([1, 1536], F32R, name="wb")
            cosPt = pw.tile([128, S], F32, name="cosPt")
            sinAt = pw.tile([128, S], F32, name="sinAt")
            nc.sync.dma_start(cosPt[:], cosP[:])
            nc.sync.dma_start(sinAt[:], sinA[:])
            for kc in range(KC):
                nc.sync.dma_start(wts[:, kc], wT[128 * kc:128 * (kc + 1)])
            for kc in range(KC):
                nc.sync.dma_start(xts[:, kc], xT[128 * kc:128 * (kc + 1)])
            nc.sync.dma_start(xone[:], xT[1024:1025])
            nc.sync.dma_start(wb[:], wT[1024:1025])

            # q/k groups: out [dout 128, n-chunk]; rope on spatial chunks
            FQK = [(0, 2), (2, 514), (514, 1026)]
            for qk in range(2):          # 0=q, 1=k
                dst = qT if qk == 0 else kT
                for g in range(4):
                    gc = qk * 512 + g * 128
                    for (f0, f1) in FQK:
                        fw = f1 - f0
                        ps = psum.tile([128, 512], F32, tag="lgt", name="psqk", bufs=2)
                        for kc in range(KC):
                            nc.tensor.matmul(
                                ps[:, :fw], wts[:, kc, gc:gc + 128],
                                xts[:, kc, f0:f1], start=(kc == 0), stop=False)
                        nc.tensor.matmul(
                            ps[:, :fw], wb[:, gc:gc + 128], xone[:, f0:f1],
                            start=False, stop=True)
                        if f0 == 0:      # token cols: no rope, plain copy
                            nc.scalar.activation(dst[:, g, 0:2], ps[:, :2],
                                                 AF.Identity)
                            continue
                        sl = slice(f0 - 2, f1 - 2)
                        tmp = pr.tile([128, 512], F32, tag="rtmp", name="rtmp")
                        cq = pr.tile([128, 512], F32, tag="rcq", name="rcq")
                        sh = nc.gpsimd if shuffle_engine == "gpsimd" else nc.vector
                        for hb in (0, 64):   # two heads per chunk
                            sh.tensor_tensor(
                                out=tmp[hb:hb + 32, :], in0=ps[hb + 32:hb + 64, :],
                                in1=sinAt[hb:hb + 32, sl], op=MUL)
                            sh.tensor_tensor(
                                out=tmp[hb + 32:hb + 64, :], in0=ps[hb:hb + 32, :],
                                in1=sinAt[hb + 32:hb + 64, sl], op=MUL)
                        nc.vector.tensor_tensor(
                            out=cq[:], in0=ps[:, :fw], in1=cosPt[:, sl], op=MUL)
                        nc.vector.tensor_tensor(
                            out=dst[:, g, f0:f1], in0=cq[:], in1=tmp[:], op=ADD)

            # v groups: out [n-chunk, dout 512]
            NSL = [(0, 2)] + [(2 + 128 * i, 2 + 128 * (i + 1)) for i in range(8)]
            for si, (n0, n1) in enumerate(NSL):
                nw = n1 - n0
                ps = psum.tile([128, 512], F32, tag="lgt", name="psv", bufs=2)
                for kc in range(KC):
                    nc.tensor.matmul(ps[:nw, :], xts[:, kc, n0:n1],
                                     wts[:, kc, 1024:1536],
                                     start=(kc == 0), stop=False)
                nc.tensor.matmul(ps[:nw, :], xone[:, n0:n1], wb[:, 1024:1536],
                                 start=False, stop=True)
                if si == 0:
                    vtk2 = pr.tile([2, HPC, 64], F32R, tag="vtk2", name="vtk2")
                    nc.scalar.activation(
                        vtk2[:],
                        ps[0:2, :].rearrange("p (h d) -> p h d", h=HPC),
                        AF.Identity)
                    nc.sync.dma_start(vtok0[:, :, 0:64], vtk2[0:1])
                    nc.sync.dma_start(vtok1[:, :, 0:64], vtk2[1:2])
                else:
                    nc.scalar.activation(
                        vz[:, si - 1, :, 0:64],
                        ps[:, :].rearrange("p (h d) -> p h d", h=HPC),
                        AF.Identity)

        if dbg:
            nc.sync.dma_start(dbg["d_qT"], qT[:])
            nc.sync.dma_start(dbg["d_kT"], kT[:])
            nc.sync.dma_start(dbg["d_vz"], vz[:])
        # ---------------- stage C: attention ----------------
        with tc.tile_pool(name="pc" + sfx, bufs=1) as pc, \
             tc.tile_pool(name="pe2" + sfx, bufs=2) as pe2:
            zt = pc.tile([32, PW], F32, name="zt")
            e0t = pc.tile([32, PW], F32, name="e0t")
            e1t = pc.tile([32, PW], F32, name="e1t")
            r0t = pc.tile([32, PW], F32, name="r0t")
            r1t = pc.tile([32, PW], F32, name="r1t")
            sgs = pc.tile([32, PW], F32, name="sgs")
            sgg0 = pc.tile([32, PW], F32R, name="sgg0")
            sgg1 = pc.tile([32, PW], F32R, name="sgg1")
            avsb = pc.tile([65, HPC, PN], F32, name="avsb")
            nc.gpsimd.memset(avsb[:, :, 1026:PN], 1.0)

            for h in range(HPC):
                j, po = h // 2, (h % 2) * 64
                etokh = pe2.tile([2, PN], F32, tag="etokh", name="etokh")
                nc.vector.memset(etokh[:, 1026:PN], 1.0)
                for qc in range(3):
                    q0 = qc * QCW
                    qsl = slice(q0, q0 + QCW)
                    et = pe2.tile([128, KC, QCW], MDT, tag="et", name="et", bufs=4)
                    for ktg in range(4):     # kt pairs
                        lg = psum.tile([128, 2, 512], F32, tag="lgt", name="lg", bufs=2)
                        for u in range(2):
                            kt = 2 * ktg + u
                            ksl = slice(2 + kt * 128, 2 + (kt + 1) * 128)
                            nc.tensor.matmul(
                                lg[:, u, :QCW], kT[po:po + 64, j, ksl],
                                qT[po:po + 64, j, qsl], start=True, stop=True)
                        nc.scalar.activation(
                            et[:, 2 * ktg:2 * ktg + 2, :], lg[:, :, :QCW],
                            AF.Exp, scale=SCALE)
                    av = psum.tile([65, 512], F32, tag="av", name="av", bufs=3)
                    for kt in range(KC):
                        nc.tensor.matmul(av[:, :QCW], vz[:, kt, h, :],
                                         et[:, kt, :], start=(kt == 0),
                                         stop=(kt == KC - 1))
                    pstokh = psum.tile([2, 512], F32, tag="tok",
                                       name="pstokh")
                    nc.tensor.matmul(
                        pstokh[:, 0:QCW], kT[po:po + 64, j, 0:2],
                        qT[po:po + 64, j, qsl], start=True, stop=True)
                    nc.scalar.activation(etokh[:, qsl], pstokh[:, 0:QCW],
                                         AF.Exp, scale=SCALE)
                    # evacuate av (A rows + Z row) to SBUF in one op
                    nc.scalar.activation(avsb[:, h, qsl], av[0:65, :QCW],
                                         AF.Identity)
                if safe:
                    for c4 in range(4):
                        nc.sync.dma_start(zt[h + 8 * c4:h + 8 * c4 + 1, :],
                                          avsb[64:65, h, PW * c4:PW * (c4 + 1)])
                else:
                    nc.sync.dma_start(zt[h::8, :], avsb[64:65, h, :])
                if safe:
                    for c4 in range(4):
                        nc.sync.dma_start(e0t[h + 8 * c4:h + 8 * c4 + 1, :],
                                          etokh[0:1, PW * c4:PW * (c4 + 1)])
                        nc.sync.dma_start(e1t[h + 8 * c4:h + 8 * c4 + 1, :],
                                          etokh[1:2, PW * c4:PW * (c4 + 1)])
                else:
                    nc.sync.dma_start(e0t[h::8, :], etokh[0:1, :])
                    nc.sync.dma_start(e1t[h::8, :], etokh[1:2, :])

            # batched corrections (all heads, full width)
            nc.vector.tensor_tensor(out=r0t[:], in0=e0t[:], in1=zt[:], op=ADD)
            nc.vector.tensor_tensor(out=r1t[:], in0=e1t[:], in1=zt[:], op=ADD)
            nc.vector.reciprocal(r0t[:], r0t[:])
            nc.vector.reciprocal(r1t[:], r1t[:])
            sc = pc.tile([32, PW], F32, name="sc")
            nc.vector.tensor_tensor(out=sc[:], in0=r0t[:], in1=r1t[:], op=ADD)
            nc.vector.tensor_scalar_mul(sgs[:], sc[:], 0.5)
            nc.vector.tensor_tensor(out=sc[:], in0=e0t[:], in1=r0t[:], op=MUL)
            nc.vector.tensor_scalar_mul(sgg0[:], sc[:], 0.5)
            nc.vector.tensor_tensor(out=sc[:], in0=e1t[:], in1=r1t[:], op=MUL)
            nc.vector.tensor_scalar_mul(sgg1[:], sc[:], 0.5)
            # token-query fixups (cols 0,1): full r_t / e00*r_t, zero cross-t
            for t, (sgg, rbs, zbs) in enumerate(
                    [(sgg0, r0t, e0t), (sgg1, r1t, e1t)]):
                tc_ = slice(t, t + 1)
                nc.vector.tensor_copy(sgs[0:8, tc_], rbs[0:8, tc_])
                nc.vector.tensor_tensor(out=sgg[0:8, tc_], in0=zbs[0:8, tc_],
                                        in1=rbs[0:8, tc_], op=MUL)
                oth = sgg1 if t == 0 else sgg0
                nc.vector.tensor_scalar_mul(oth[0:8, tc_], oth[0:8, tc_], 0.0)

            # final combine per head
            for h in range(HPC):
                j, po = h // 2, (h % 2) * 64
                sgsr = pe2.tile([1, PN], F32, tag="sgsr", name="sgsr")
                if safe:
                    for c4 in range(4):
                        nc.sync.dma_start(sgsr[0:1, PW * c4:PW * (c4 + 1)],
                                          sgs[h + 8 * c4:h + 8 * c4 + 1, :])
                else:
                    nc.sync.dma_start(sgsr[:], sgs[h::8, :])
                bc = pe2.tile([64, PN], F32, tag="bc", name="bc")
                nc.gpsimd.partition_broadcast(bc[:], sgsr[0:1, :])
                sgr0 = pe2.tile([1, PN], F32R, tag="sgr0", name="sgr0")
                sgr1 = pe2.tile([1, PN], F32R, tag="sgr1", name="sgr1")
                if safe:
                    for c4 in range(4):
                        nc.sync.dma_start(sgr0[0:1, PW * c4:PW * (c4 + 1)],
                                          sgg0[h + 8 * c4:h + 8 * c4 + 1, :])
                        nc.sync.dma_start(sgr1[0:1, PW * c4:PW * (c4 + 1)],
                                          sgg1[h + 8 * c4:h + 8 * c4 + 1, :])
                else:
                    nc.sync.dma_start(sgr0[:], sgg0[h::8, :])
                    nc.sync.dma_start(sgr1[:], sgg1[h::8, :])
                for qc in range(3):
                    q0 = qc * QCW
                    qsl = slice(q0, q0 + QCW)
                    vt = psum.tile([64, 512], F32, tag="av", name="vt", bufs=3)
                    nc.tensor.matmul(vt[:, :QCW], vtok0[:, h, :64],
                                     sgr0[:, qsl], start=True, stop=False)
                    nc.tensor.matmul(vt[:, :QCW], vtok1[:, h, :64],
                                     sgr1[:, qsl], start=False, stop=True)
                    tf = pe2.tile([64, 512], F32, tag="tf", name="tf")
                    nc.vector.tensor_tensor(out=tf[:, :QCW],
                                            in0=avsb[0:64, h, qsl],
                                            in1=bc[:, qsl], op=MUL)
                    nc.vector.tensor_tensor(out=oT[po:po + 64, j, qsl],
                                            in0=vt[:, :QCW], in1=tf[:, :QCW],
                                            op=ADD)

            if dbg:
                nc.sync.dma_start(dbg["d_avsb"], avsb[:])
                nc.sync.dma_start(dbg["d_zb"][:, 0], zt[:])
                nc.sync.dma_start(dbg["d_zb"][:, 1], e0t[:])
                nc.sync.dma_start(dbg["d_zb"][:, 2], e1t[:])
                nc.sync.dma_start(dbg["d_sgs"], sgs[:])
                nc.sync.dma_start(dbg["d_sgg0"], sgg0[:])
        if dbg:
            nc.sync.dma_start(dbg["d_oT"], oT[:])
        # ---------------- stage D: output projection ----------------
        with tc.tile_pool(name="pd" + sfx, bufs=1) as pd, \
             tc.tile_pool(name="po2" + sfx, bufs=3) as po2:
            pjts = pd.tile([128, 4, 1024], F32R, name="pjts")
            for j in range(4):
                nc.sync.dma_start(pjts[:, j], pjT[128 * j:128 * (j + 1)])
            NSL = [(128 * i, 128 * (i + 1)) for i in range(8)] + [(1024, 1026)]
            for (n0, n1) in NSL:
                nw = n1 - n0
                for f in range(2):
                    ps = psum.tile([128, 512], F32, tag="lgt", name="psp", bufs=2)
                    for j in range(4):
                        nc.tensor.matmul(
                            ps[:nw, :], oT[:, j, n0:n1],
                            pjts[:, j, 512 * f:512 * (f + 1)],
                            start=(j == 0), stop=(j == 3))
                    ob = po2.tile([128, 512], F32, tag="ob", name="ob")
                    nc.vector.tensor_copy(ob[:nw, :], ps[:nw, :])
                    nc.sync.dma_start(out[n0:n1, 512 * f:512 * (f + 1)],
                                      ob[:nw, :])


# ---------------- host side ----------------

_PERM = np.concatenate([np.arange(0, 64, 2), np.arange(1, 64, 2)])


def _host_prep(x, rope, qkv_w, q_bias, k_bias, v_bias, proj_w):
    """Build per-core input dicts."""
    x = np.asarray(x, np.float32)
    rope = np.asarray(rope, np.float32)
    qkv_w = np.asarray(qkv_w, np.float32)
    q_bias = np.asarray(q_bias, np.float32)
    k_bias = np.asarray(k_bias, np.float32)
    v_bias = np.asarray(v_bias, np.float32)
    proj_w = np.asarray(proj_w, np.float32)

    sin = rope[:, :D].T          # [64, S]
    cos = rope[:, D:].T
    cos64 = cos[_PERM]
    sinA64 = np.empty((64, S), np.float32)
    sinA64[0:32] = -sin[0::2]
    sinA64[32:64] = sin[1::2]
    cosP = np.vstack([cos64, cos64]).astype(np.float32)
    sinA = np.vstack([sinA64, sinA64]).astype(np.float32)

    in_maps = []
    for core in range(NCORES):
        b, hh = core // 2, core % 2
        hs = hh * 512
        idx = np.concatenate([h * 64 + _PERM for h in range(HPC)]) + hs
        wq = qkv_w[0:C][idx]
        wk = qkv_w[C:2 * C][idx]
        wv = qkv_w[2 * C:3 * C][hs:hs + 512]
        W3 = np.concatenate([wq, wk, wv], 0)          # [1536, 1024]
        wTa = np.empty((1025, 1536), np.float32)
        wTa[0:1024] = W3.T
        wTa[1024] = np.concatenate(
            [q_bias[idx], k_bias[idx], v_bias[hs:hs + 512]])
        xTa = np.empty((1025, N), np.float32)
        xTa[0:1024] = x[b].T
        xTa[1024] = 1.0
        pjTa = np.ascontiguousarray(proj_w[:, hs:hs + 512].T)  # [512, 1024]
        in_maps.append({"xT": xTa, "wT": wTa, "pjT": pjTa,
                        "cosP": cosP, "sinA": sinA})
    return in_maps


def _get_runner():
    return _get_runner_rep(1)


def _get_runner_rep(repeat, serial=False):
    key = f"runner{repeat}s{int(serial)}"
    if key in _CACHE:
        return _CACHE[key]
    import jax
    from jax.sharding import Mesh, PartitionSpec
    from jax.experimental.shard_map import shard_map
    import concourse.mybir as mybir
    from concourse import bass2jax

    nc = _build_nc(repeat=repeat, serial=serial)
    bass2jax.install_neuronx_cc_hook()
    in_names, out_names, out_avals = [], [], []
    partition_name = (nc.partition_id_tensor.name
                      if nc.partition_id_tensor else None)
    for alloc in nc.m.functions[0].allocations:
        if not isinstance(alloc, mybir.MemoryLocationSet):
            continue
        name = alloc.memorylocations[0].name
        if alloc.kind == "ExternalInput":
            if name != partition_name:
                in_names.append(name)
        elif alloc.kind == "ExternalOutput":
            out_names.append(name)
            out_avals.append(jax.core.ShapedArray(
                tuple(alloc.tensor_shape), mybir.dt.np(alloc.dtype)))
    all_in = list(in_names) + list(out_names)
    if partition_name is not None:
        all_in.append(partition_name)

    def _body(*args):
        operands = list(args)
        if partition_name is not None:
            operands.append(bass2jax.partition_id_tensor())
        return tuple(bass2jax._bass_exec_p.bind(
            *operands, out_avals=tuple(out_avals), in_names=tuple(all_in),
            out_names=tuple(out_names), lowering_input_output_aliases=(),
            sim_require_finite=True, sim_require_nnan=True, nc=nc))

    mesh = Mesh(np.asarray(jax.devices()[:NCORES]), ("core",))
    nin = len(in_names)
    nout = len(out_names)
    fn = jax.jit(
        shard_map(_body, mesh=mesh,
                  in_specs=(PartitionSpec("core"),) * (nin + nout),
                  out_specs=(PartitionSpec("core"),) * nout,
                  check_rep=False),
        keep_unused=True)
    _CACHE[key] = (fn, mesh, in_names, out_names, out_avals)
    return _CACHE[key]


def kernel(x, rope, qkv_w, q_bias, k_bias, v_bias, proj_w, proj_b):
    import jax
    from jax.sharding import PartitionSpec
    fn, mesh, in_names, out_names, out_avals = _get_runner()
    in_maps = _host_prep(x, rope, qkv_w, q_bias, k_bias, v_bias, proj_w)
    sharding = jax.sharding.NamedSharding(mesh, PartitionSpec("core"))
    args = []
    for name in in_names:
        cat = np.concatenate([m[name] for m in in_maps], axis=0)
        args.append(jax.device_put(cat, sharding))
    for av in out_avals:
        z = np.zeros((NCORES * av.shape[0], *av.shape[1:]), av.dtype)
        args.append(jax.device_put(z, sharding))
    outs = fn(*args)
    parts = np.asarray(outs[out_names.index("out")]).reshape(
        NCORES, N, C)
    proj_b = np.asarray(proj_b, np.float32)
    res = np.empty((B, N, C), np.float32)
    for b in range(B):
        res[b] = parts[2 * b] + parts[2 * b + 1] + proj_b
    return res
